# revision 4
# baseline (speedup 1.0000x reference)
"""TopK sparse autoencoder forward on 8 TRN2 NeuronCores — v3.

Data-parallel over the batch (1024 rows/core, no collectives).
Per core, 4 superpasses of 256 rows (2 row-tiles of 128):
  encode : pre_acts = xT.T @ W_enc as a SINGLE fp32r matmul stream.
           The PE runs fp32r at bf16 rate for moving dim >= 256; fp32r
           rounds each operand to 11 mantissa bits (RNE).  x and W_enc
           are pre-rounded to that grid on the host so the HW rounding
           is a no-op and the selection error is exactly the simulated
           ~1.5e-2 (< 2e-2 gate).  f32 PSUM -> f32 pre_acts in SBUF.
  topk   : per-128-window top-8 candidates (InstMax, interleaved into the
           encode stream) -> 13-round max/match_replace cascade -> per-row
           threshold t = 100th largest value
  mask   : encoded = pre_acts * (pre_acts >= t), bf16 into a bitcast
           overlay of the pre_acts tiles
  transp : PE transposes of 128x128 blocks -> encodedT overlay
  decode : x_hat = encodedT.T @ W_dec (bf16), PSUM accumulated over d_sae

pre_acts for each row-tile is held as TWO SBUF tiles (paA = d_sae lower
half, which also carries the bf16 enc overlay; paB = upper half carrying
the encodedT overlay) so the next superpass's encode (which first fills
paA) can start while the current superpass's decode is still reading
paB's encodedT.

All DRAM tensors are pre-packed on the host into the exact contiguous
[128, free] tile layouts the kernel consumes, so every DMA is a dense
partition-major block (1-2 MiB).
"""

import numpy as np
import ml_dtypes

import concourse.mybir as mybir
from concourse.bass import Bass
from concourse.bass_utils import run_bass_kernel_spmd

import bass_rust
from concourse.tile import TileContext, ScopedClock

# This walrus build rejects instructions carrying more than a couple of
# sem waits ("Too many sync wait commands"), which Tile's scheduler and
# tail drain freely emit.  PatchedTileContext re-emits the tail-drain
# waits as single-wait sync.wait_ge instructions; fix_sync_waits() walks
# the finished module and hoists excess waits off any instruction onto
# same-engine NOPs inserted before it.
MAX_WAITS = 1


class PatchedTileContext(TileContext):
    def _drain_and_barrier(self, tick_clock, wait_clock):
        probe = self.nc.sync.drain()
        wait_clock.add_sem_waits(
            probe.ins, ScopedClock({None: tick_clock.global_clock})
        )
        waits = list(probe.ins.sync_info.on_wait)
        probe.ins.sync_info = bass_rust.SyncInfo(on_wait=[], on_update=[])
        assert self.sems is not None
        handles = {h.num: h for h in self.sems.allocated().values()}
        for w in waits:
            sem = handles.get(w.id)
            assert sem is not None, f"no handle for sem {w.ant_name}"
            assert w.wait_mode == "sem-ge-imm", w.wait_mode
            self.nc.sync.wait_ge(sem, w.wait_value)
        self.nc.all_engine_barrier()
        popped = self.nc._tile_sem_poison_stack.pop()
        assert popped is self._sem_poison
        self.nc.clear_and_free_semaphores(list(self.sems.allocated().values()))
        self.nc.all_engine_barrier()


def fix_sync_waits(nc, max_waits=MAX_WAITS):
    ctr = 0
    for f in nc.m.functions:
        for bb in f.blocks:
            insts = list(bb.instructions)
            out, changed = [], False
            for inst in insts:
                si = inst.sync_info
                if si is not None and len(si.on_wait) > max_waits:
                    waits = list(si.on_wait)
                    head, tail = waits[:-max_waits], waits[-max_waits:]
                    for j in range(0, len(head), max_waits):
                        nop = mybir.InstNoOp(
                            name=f"I-waitfix-{ctr}", engine=inst.engine)
                        ctr += 1
                        nop.sync_info = bass_rust.SyncInfo(
                            on_wait=head[j:j + max_waits], on_update=[])
                        out.append(nop)
                    inst.sync_info = bass_rust.SyncInfo(
                        on_wait=tail, on_update=list(si.on_update))
                    changed = True
                out.append(inst)
            if changed:
                bb.instructions = out


F32 = mybir.dt.float32
F32R = mybir.dt.float32r
BF16 = mybir.dt.bfloat16

D_IN = 2048
D_SAE = 16384
HSAE = D_SAE // 2                 # 8192, one pre_acts half
K_TOP = 100
B = 8192
N_CORES = 8
ROWS_PER_CORE = B // N_CORES      # 1024
SP_ROWS = 256                     # rows per superpass
N_SP = ROWS_PER_CORE // SP_ROWS   # 4
NTILE = SP_ROWS // 128            # 2 row-tiles of 128 rows
JG = 256                          # encode d_sae group width
N_JG = D_SAE // JG                # 64
WIN = 128                         # L1 window width
NR = 13                           # cascade rounds: 13*8 = 104 >= 100
NCAND = 8 * (D_SAE // WIN)        # 1024
DS = 512                          # decode d_sae slab rows
N_DS = D_SAE // DS                # 32
NEG = -1.0e30


def build_nc(kch: int, phases=("e", "t", "d")) -> Bass:
    """kch: number of 128-row contraction chunks (16, or 17 with bias row)."""
    nc = Bass()
    xts_d = nc.declare_dram_parameter(
        "xts", [N_SP, 128, kch * SP_ROWS], F32R, isOutput=False)
    we = nc.declare_dram_parameter(
        "we", [N_JG, 128, kch * JG], F32R, isOutput=False)
    wd = nc.declare_dram_parameter(
        "wd", [2, N_DS, 128, (DS // 128) * 1024], BF16, isOutput=False)
    ident = nc.declare_dram_parameter("ident", [128, 128], BF16, isOutput=False)
    y = nc.declare_dram_parameter("y", [ROWS_PER_CORE, D_IN], F32, isOutput=True)

    with PatchedTileContext(nc) as tc:
        with (
            tc.tile_pool(name="paA", bufs=2) as paA_pool,
            tc.tile_pool(name="paB", bufs=2) as paB_pool,
            tc.tile_pool(name="wep", bufs=2) as we_pool,
            tc.tile_pool(name="wdp", bufs=2) as wd_pool,
            tc.tile_pool(name="xtp", bufs=1) as xt_pool,
            tc.tile_pool(name="cand", bufs=2) as cand_pool,
            tc.tile_pool(name="m8", bufs=2) as m8_pool,
            tc.tile_pool(name="const", bufs=1) as const_pool,
            tc.tile_pool(name="outp", bufs=1) as out_pool,
            tc.tile_pool(name="pse", bufs=2, space="PSUM") as psum_e,
            tc.tile_pool(name="pst", bufs=2, space="PSUM") as psum_t,
            tc.tile_pool(name="psd", bufs=2, space="PSUM") as psum_d,
        ):
            identity = const_pool.tile([128, 128], BF16, name="identity")
            nc.sync.dma_start(out=identity, in_=ident[:, :])

            for sp in range(N_SP):
                r0 = sp * SP_ROWS
                xts = xt_pool.tile([128, kch, SP_ROWS], F32R, tag="xts",
                                   name="xts")
                nc.sync.dma_start(out=xts, in_=xts_d[sp])
                # pre_acts halves: A = d_sae [0, 8192), B = [8192, 16384)
                paA = [paA_pool.tile([128, HSAE], F32, tag="paA", name="paA")
                       for _ in range(NTILE)]
                paB = [paB_pool.tile([128, HSAE], F32, tag="paB", name="paB")
                       for _ in range(NTILE)]
                cands = [cand_pool.tile([128, NCAND], F32, tag="cand",
                                        name="cand") for _ in range(NTILE)]

                def pa_slice(b, j0, width):
                    """f32 view of pre_acts[b][:, j0:j0+width] within a half"""
                    if j0 < HSAE:
                        assert j0 + width <= HSAE
                        return paA[b][:, j0:j0 + width]
                    return paB[b][:, j0 - HSAE:j0 - HSAE + width]

                # ---------- encode: single fp32r matmul stream ----------
                for jg in range(N_JG):
                    wes = we_pool.tile([128, kch, JG], F32R, tag="wes",
                                       name="wes")
                    nc.sync.dma_start(out=wes, in_=we[jg])
                    for b in range(NTILE):
                        ps = psum_e.tile([128, JG], F32, tag="pse", name="pse")
                        for k in range(kch):
                            nc.tensor.matmul(
                                ps,
                                lhsT=xts[:, k, b * 128:(b + 1) * 128],
                                rhs=wes[:, k, :],
                                start=(k == 0), stop=(k == kch - 1))
                        nc.scalar.copy(out=pa_slice(b, jg * JG, JG), in_=ps)
                        if "t" in phases:
                            # L1 top-8 of each 128-window, as chunks land
                            for w in range(JG // WIN):
                                wi = jg * (JG // WIN) + w
                                nc.vector.max(
                                    out=cands[b][:, wi * 8:(wi + 1) * 8],
                                    in_=pa_slice(b, jg * JG + w * WIN, WIN),
                                )

                # ---------- topk threshold + mask + transpose ----------
                encT = []  # bf16 views of paB holding encodedT
                for b in range(NTILE if "t" in phases else 0):
                    # L2: extract top-104 of the 1024 candidates
                    m8 = m8_pool.tile([128, NR * 8], F32, tag="m8", name="m8")
                    cur = cands[b]
                    for r in range(NR):
                        nc.vector.max(out=m8[:, r * 8:(r + 1) * 8], in_=cur)
                        if r < NR - 1:
                            nc.vector.match_replace(
                                out=cur,
                                in_to_replace=m8[:, r * 8:(r + 1) * 8],
                                in_values=cur,
                                imm_value=NEG,
                            )
                    t_ap = m8[:, K_TOP - 1:K_TOP]  # 100th largest

                    # mask: encoded = pre * (pre >= t) as bf16 into paA's
                    # bf16 overlay; low half sourced from paA, high from paB.
                    # high_priority: keep the masks ahead of the next tile's
                    # cascade in the DVE stream so transposes start earlier.
                    encv = paA[b].bitcast(BF16)   # [128, 16384] bf16
                    with tc.high_priority(offset=2000):
                        for half, src in ((0, paA[b]), (1, paB[b])):
                            nc.vector.scalar_tensor_tensor(
                                out=encv[:, half * HSAE:(half + 1) * HSAE],
                                in0=src,
                                scalar=t_ap,
                                in1=src,
                                op0=mybir.AluOpType.is_ge,
                                op1=mybir.AluOpType.mult,
                            )
                    # PE-transpose 128x128 blocks into encT (paB overlay);
                    # 8 transposes batch into one [128,1024] PSUM tile,
                    # drained by a single Scalar copy
                    etv = paB[b].bitcast(BF16)    # [128, 16384] bf16
                    for g in range(D_SAE // 1024):   # 16 groups of 8 blocks
                        c0 = g * 8
                        pt = psum_t.tile([128, 1024], BF16, tag="pst",
                                         name="pst")
                        for i in range(8):
                            c = c0 + i
                            nc.tensor.transpose(
                                out=pt[:, i * 128:(i + 1) * 128],
                                in_=encv[:, c * 128:(c + 1) * 128],
                                identity=identity,
                            )
                        nc.scalar.copy(
                            out=etv[:, c0 * 128:(c0 + 8) * 128], in_=pt)
                    encT.append(etv)

                # ---------- decode ----------
                for h in range(2 if "d" in phases else 0):  # d_in halves
                    pd = [psum_d.tile([128, 1024], F32, tag="psd", name="psd")
                          for _ in range(NTILE)]
                    for ds in range(N_DS):
                        wds = wd_pool.tile([128, DS // 128, 1024], BF16,
                                           tag="wds", name="wds")
                        nc.sync.dma_start(out=wds, in_=wd[h, ds])
                        for b in range(NTILE):
                            for c in range(DS // 128):
                                kc = ds * (DS // 128) + c
                                for n in range(2):
                                    nc.tensor.matmul(
                                        pd[b][:, n * 512:(n + 1) * 512],
                                        lhsT=encT[b][:, kc * 128:(kc + 1) * 128],
                                        rhs=wds[:, c, n * 512:(n + 1) * 512],
                                        start=(kc == 0),
                                        stop=(kc == D_SAE // 128 - 1),
                                    )
                    for b in range(NTILE):
                        osb = out_pool.tile([128, 1024], F32, tag="osb",
                                            name="osb")
                        nc.scalar.copy(out=osb, in_=pd[b])
                        nc.sync.dma_start(
                            out=y[r0 + b * 128:r0 + (b + 1) * 128,
                                  h * 1024:(h + 1) * 1024],
                            in_=osb,
                        )
    return nc


def _round11(a):
    """Round f32 to fp32r's 11-bit mantissa grid (round-to-nearest)."""
    u = a.astype(np.float32).view(np.uint32).astype(np.uint64)
    shift = 23 - 11
    r = ((u + (1 << (shift - 1))) >> shift) << shift
    return r.astype(np.uint32).view(np.float32)


def _prep_inputs(x, W_enc, b_enc, W_dec, b_dec):
    x_eff = x - b_dec[None, :]
    if np.any(b_enc != 0.0):
        kch = D_IN // 128 + 1
        pad = kch * 128 - D_IN - 1
        we_np = np.concatenate(
            [W_enc, b_enc[None, :], np.zeros((pad, D_SAE), np.float32)], axis=0)
        x_ext = np.concatenate(
            [x_eff, np.ones((B, 1), np.float32), np.zeros((B, pad), np.float32)],
            axis=1)
    else:
        kch = D_IN // 128
        we_np = W_enc
        x_ext = x_eff

    # we: [N_JG, 128, kch, JG] contiguous per jg block, fp32r-rounded f32
    we_t = np.ascontiguousarray(
        _round11(we_np).reshape(kch, 128, N_JG, JG).transpose(2, 1, 0, 3)
        .reshape(N_JG, 128, kch * JG))

    # wd: [2, N_DS, 128, DS//128, 1024] bf16
    wd_bf = W_dec.astype(ml_dtypes.bfloat16)
    wd_t = wd_bf.reshape(N_DS, DS // 128, 128, 2, 1024).transpose(3, 0, 2, 1, 4)
    wd_t = np.ascontiguousarray(wd_t.reshape(2, N_DS, 128, (DS // 128) * 1024))

    ident = np.eye(128, dtype=ml_dtypes.bfloat16)

    in_maps = []
    for i in range(N_CORES):
        rows = x_ext[i * ROWS_PER_CORE:(i + 1) * ROWS_PER_CORE]
        xt = np.ascontiguousarray(rows.T).astype(np.float32)  # [d_in_pad, 1024]
        xts = np.ascontiguousarray(
            _round11(xt).reshape(kch, 128, N_SP, SP_ROWS).transpose(2, 1, 0, 3)
            .reshape(N_SP, 128, kch * SP_ROWS))
        in_maps.append({"xts": xts, "we": we_t, "wd": wd_t, "ident": ident})
    return kch, in_maps


LAST_RESULT = None


def kernel(x, W_enc, b_enc, W_dec, b_dec):
    global LAST_RESULT
    x = np.asarray(x, np.float32)
    W_enc = np.asarray(W_enc, np.float32)
    b_enc = np.asarray(b_enc, np.float32)
    W_dec = np.asarray(W_dec, np.float32)
    b_dec = np.asarray(b_dec, np.float32)
    kch, in_maps = _prep_inputs(x, W_enc, b_enc, W_dec, b_dec)
    nc = build_nc(kch)
    fix_sync_waits(nc)
    res = run_bass_kernel_spmd(nc, in_maps, list(range(N_CORES)))
    LAST_RESULT = res
    out = np.concatenate([res.results[i]["y"] for i in range(N_CORES)], axis=0)
    if np.any(b_dec != 0.0):
        out = out + b_dec[None, :]
    return out
